# revision 37
# baseline (speedup 1.0000x reference)
"""Fused LayerNorm + causal multi-head attention for Trainium2, 8 NeuronCores.

Problem: x[2,2048,1024] -> LN -> qkv proj (w_qkv[1024,3072]) -> 16-head causal
attention (d=64) -> out proj (w_out[1024,1024]).

Sharding (no cross-core communication):
  core c = b*4 + hg   (b in {0,1} batches, hg in {0..3} head-groups of 4 heads)
  Each core computes its batch's LN + its 4 heads' qkv/attention + a partial
  out-projection (its 256 rows of w_out). Host sums the 4 partials per batch.

Perf design (v2, ~231us vs 358us baseline): every matmul is shaped "fat"
(K=128 streamed rows, M<=128 with padding) so the PE HAM clock gate stays at
2.4 GHz (the baseline's attention ran entirely at the cold 1.2 GHz clock
because its K=64/M=66 matmuls never register enough array activity):
  - s = k.T @ q uses zero-padded k tiles (kTz): head-even data on partitions
    0:64 with zeros below, head-odd data on 64:128 with zeros above, so the
    shared q tile (both heads stacked) streams through a K=128 matmul.
  - PV uses v_pad[128,128] stationary tiles: head-even v at cols 0:64 with a
    ones col at 64 (softmax denominator lands on psum row 64); head-odd v at
    cols 64:128 with a ones col at 0 (denominator on psum row 0). Outputs land
    lane-aligned with the oT layout (even head rows 0:64, odd rows 64:128), so
    normalization is pure DVE (no partition-shift DMA).
  - causal masks are ADDITIVE pre-exp, accumulated into the score psum by a
    (-30000*triu).T @ I matmul: the mask stays inside the PE stream instead
    of adding a GpSimd hop between exp and PV.
  - i-blocks run in order 3,2,1,0 and each block's out-projection is deferred
    and drip-fed into the NEXT block's j-loop as PE filler: the exp-bound
    steady state would otherwise leave ~25% distributed PE idle, which trips
    the HAM activity monitor back to 1.2 GHz.
  - exp is one [128,1024] ACT instruction per head-pair j-tile (2-bank psum
    read); bf16 out-projection output halves the store DMA bytes.
PSUM (8 banks): s/rb/outproj share one 3-buf [128,1024] pool (6) + o0/o1 (2).
Known HW constraints honored: reciprocal_approx_fast must be base-partition-0
and not in-place; ACT ops need quad-aligned partition bases; f32r matmul
operands must be produced as f32r (rounded) by the writing op; K=1 f32r
matmuls only from partition 0/64; GPSIMD cannot touch PSUM.
"""
import os
import sys

for _p in ("/opt/trn_rl_repo", "/root/.axon_site/_ro/trn_rl_repo"):
    if os.path.isdir(_p) and _p not in sys.path:
        sys.path.insert(0, _p)

import numpy as np

import concourse.bass as bass  # noqa: F401  (import side effects / debugging)
import concourse.mybir as mybir
import concourse.tile as tile
from concourse import bacc
from concourse.bass_utils import run_bass_kernel_spmd

F32 = mybir.dt.float32
F32R = mybir.dt.float32r
BF16 = mybir.dt.bfloat16
MUL = mybir.AluOpType.mult
ADD = mybir.AluOpType.add
SUB = mybir.AluOpType.subtract
AF = mybir.ActivationFunctionType

B, N, DIM = 2, 2048, 1024
HEADS, DH = 16, 64
HPC = 4            # heads per core
CD = HPC * DH      # 256 output channels per core
SCALE = DH ** -0.5
EPS = 1e-5
NT = N // 512      # 4 col-blocks of 512
NK = DIM // 128    # 8 contraction chunks
NROW = N // 128    # 16 row tiles of 128

# w layout cols: [q01 q23 k01 k23 v01 v23] blocks of 128; emit v first so the
# PE-transposes interleave with later q/k matmuls (keeps HAM warm).
CT_V = (4, 5)
CT_ORDER = (4, 5, 0, 2, 1, 3)   # v01 v23 q01 k01 q23 k23


def _r32(x, bits=13):
    """tf32-style rounding: round-to-nearest, drop low `bits` mantissa bits."""
    v = np.ascontiguousarray(x, dtype=np.float32).view(np.uint32)
    v = (v + (1 << (bits - 1))) & np.uint32(~((1 << bits) - 1) & 0xFFFFFFFF)
    return v.view(np.float32)


def _build():
    nc = bacc.Bacc("TRN2", target_bir_lowering=False, debug=False)

    xT_ext = nc.declare_dram_parameter("xT", [DIM, N], BF16, isOutput=False)
    w_ext = nc.declare_dram_parameter("wqkv", [DIM, 3 * CD], BF16, isOutput=False)
    uv_ext = nc.declare_dram_parameter("uv", [2, 3 * CD], F32R, isOutput=False)
    wo_ext = nc.declare_dram_parameter("wout", [CD, DIM], BF16, isOutput=False)
    ones_ext = nc.declare_dram_parameter("ones", [128, 128], F32R, isOutput=False)
    stri_ext = nc.declare_dram_parameter("stri", [128, 128], BF16, isOutput=False)
    id_ext = nc.declare_dram_parameter("ident", [128, 128], BF16, isOutput=False)
    out_ext = nc.declare_dram_parameter("out", [N, DIM], BF16, isOutput=True)

    with tile.TileContext(nc) as tc:
        with (
            nc.allow_low_precision(reason="float32r is 4-byte; psum stays f32"),
            tc.tile_pool(name="persist", bufs=1) as pp,
            tc.tile_pool(name="small", bufs=1) as sp,
        ):
            ones_t = pp.tile([128, 128], F32R, tag="ones")
            stri_t = pp.tile([128, 128], BF16, tag="stri")
            ident_t = pp.tile([128, 128], BF16, tag="ident")
            nc.sync.dma_start(ones_t[:], ones_ext[:])
            nc.sync.dma_start(stri_t[:], stri_ext[:])
            nc.sync.dma_start(ident_t[:], id_ext[:])
            ones_b = pp.tile([128, 128], BF16, tag="ones_b")
            nc.vector.tensor_scalar(ones_b[:], ones_t[:], 0.0, 1.0,
                                    op0=MUL, op1=ADD)

            # persistent SBUF tensors
            qT = [pp.tile([128, N], BF16, tag=f"qT{p}", name=f"qT{p}")
                  for p in range(2)]                      # q pair tiles
            vst = [pp.tile([128, N], BF16, tag=f"vst{p}", name=f"vst{p}")
                   for p in range(2)]                     # v staging (dh-major)
            kTz = [pp.tile([128, N], BF16, tag=f"kTz{h}", name=f"kTz{h}")
                   for h in range(4)]                     # zero-padded k tiles
            v_pad = [pp.tile([128, NROW, 128], BF16, tag=f"vp{h}",
                             name=f"vp{h}") for h in range(4)]
            oT = [pp.tile([128, N], BF16, tag=f"oT{p}", name=f"oT{p}")
                  for p in range(2)]
            a_bc = pp.tile([128, N], F32R, tag="a_bc")    # rs[n] broadcast
            rows = sp.tile([66, N], F32R, tag="rows")     # p0=-mean p1=std

            # ---------------- phase A: stats + qkv + v transpose ----------
            with (
                tc.tile_pool(name="pA", bufs=1) as pa,
                tc.tile_pool(name="pAx", bufs=2) as pax,
                tc.tile_pool(name="psA", bufs=1, space="PSUM") as psa,
                tc.tile_pool(name="psAm", bufs=3, space="PSUM") as psam,
                tc.tile_pool(name="psAv", bufs=2, space="PSUM") as psav,
            ):
                xT = pa.tile([128, NK, N], BF16, tag="xT")
                wq = pa.tile([128, NK, 3 * CD], BF16, tag="wq")
                uv_t = pa.tile([2, 3 * CD], F32R, tag="uv")
                wo_t = pa.tile([128, 2, DIM], BF16, tag="wo")
                nc.sync.dma_start(uv_t[:], uv_ext[:])
                xT_d = xT_ext[:].rearrange("(c p) n -> p c n", p=128)
                w_d = w_ext[:].rearrange("(c p) m -> p c m", p=128)
                # x loads ordered so stats(t=0) can start after the first 1 MB
                for k in range(NK):
                    nc.sync.dma_start(xT[:, k, 0:512], xT_d[:, k, 0:512])
                for k in range(NK):
                    nc.sync.dma_start(xT[:, k, 512:1024], xT_d[:, k, 512:1024])
                for k in range(NK):
                    nc.sync.dma_start(wq[:, k, :], w_d[:, k, :])
                for k in range(NK):
                    nc.sync.dma_start(xT[:, k, 1024:2048], xT_d[:, k, 1024:2048])
                wo_d = wo_ext[:].rearrange("(c p) m -> p c m", p=128)
                nc.sync.dma_start(wo_t[:, 0, :], wo_d[:, 0, :])
                nc.sync.dma_start(wo_t[:, 1, :], wo_d[:, 1, :])

                # stats per 512-col block: colsums of x and x^2 replicated to
                # all partitions via ones-matmuls; then mean/std/rs rows.
                # All row-chain ops run on partitions 0:66 (data replicated)
                # so each needed row (p0, p1, p64) is produced in-lane.
                for t in range(NT):
                    cs = slice(t * 512, (t + 1) * 512)
                    ps_s = psa.tile([128, 512], F32, tag="st_s")
                    ps_q = psa.tile([128, 512], F32, tag="st_q")
                    for k in range(NK):
                        xsq = pax.tile([128, 512], BF16, tag="xsq")
                        eng = nc.vector if k % 2 == 0 else nc.gpsimd
                        eng.tensor_tensor(xsq[:], xT[:, k, cs],
                                          xT[:, k, cs], op=MUL)
                        nc.tensor.matmul(ps_s[:], ones_b[:], xT[:, k, cs],
                                         start=(k == 0), stop=(k == NK - 1),
                                         skip_group_check=True)
                        nc.tensor.matmul(ps_q[:], ones_b[:], xsq[:],
                                         start=(k == 0), stop=(k == NK - 1),
                                         skip_group_check=True)
                    scm = pax.tile([66, 512], F32, tag="scm")
                    scq = pax.tile([66, 512], F32, tag="scq")
                    scv = pax.tile([66, 512], F32, tag="scv")
                    nc.vector.tensor_scalar(scm[:], ps_s[0:66, :], 1.0 / DIM,
                                            None, op0=MUL)
                    nc.vector.tensor_scalar(scq[:], ps_q[0:66, :], 1.0 / DIM,
                                            EPS, op0=MUL, op1=ADD)
                    nc.vector.tensor_tensor(scv[:], scm[:], scm[:], op=MUL)
                    nc.vector.tensor_tensor(scv[:], scq[:], scv[:], op=SUB)
                    # std rows at p0:2 (ACT base must be quad-aligned), then
                    # overwrite p0 with -mean: rows = [-mean@p0, std@p1]
                    nc.scalar.activation(rows[0:2, cs], scv[0:2, :], AF.Sqrt)
                    nc.vector.tensor_scalar(rows[0:1, cs], scm[0:1, :], -1.0,
                                            None, op0=MUL)
                    # rs = 1/std at p0, broadcast to all partitions via K=1.
                    # (reciprocal_approx_fast is a custom DVE op: base
                    # partition must be 0 and it must NOT run in-place.)
                    scs = pax.tile([1, 512], F32, tag="scs")
                    scs2 = pax.tile([1, 512], F32, tag="scs2")
                    scr_r = pax.tile([1, 512], F32R, tag="scr_r")
                    nc.scalar.activation(scs[0:1, :], scv[0:1, :], AF.Sqrt)
                    nc.vector.reciprocal_approx_fast(scs2[0:1, :],
                                                     scs[0:1, :])
                    nc.vector.tensor_copy(scr_r[0:1, :], scs2[0:1, :])
                    ps_ab = psa.tile([128, 512], F32, tag="ab")
                    nc.tensor.matmul(ps_ab[:], ones_t[0:1, :],
                                     scr_r[0:1, :], start=True,
                                     stop=True, skip_group_check=True)
                    nc.vector.tensor_copy(a_bc[:, cs], ps_ab[:])

                # padded-tile pre-fill AFTER the stats loop: gpsimd executes
                # in order, and ~14us of memsets ahead of the stats squares
                # would stall the x^2 colsum chain at kernel start.
                # kTz[2p]: head-even k on partitions 0:64, zeros below;
                # kTz[2p+1]: head-odd k on 64:128, zeros above.
                # v_pad[2p]: v at cols 0:64, ones col 64, zeros 65:128;
                # v_pad[2p+1]: ones col 0, zeros 1:64, v at cols 64:128.
                for p in range(2):
                    nc.gpsimd.memset(kTz[2 * p][64:128, :], 0.0)
                    nc.gpsimd.memset(kTz[2 * p + 1][0:64, :], 0.0)
                for h in range(4):
                    nc.gpsimd.memset(v_pad[h][:], 0.0)
                for p in range(2):
                    nc.gpsimd.memset(v_pad[2 * p][:, :, 64:65], 1.0)
                    nc.gpsimd.memset(v_pad[2 * p + 1][:, :, 0:1], 1.0)

                # qkv projection; v cts first, then q/k with the 32 v
                # transposes interleaved (8 after each ct) to keep PE dense.
                def _vtrans(hp, tb):
                    ps_t = psav.tile([128, 128], BF16, tag="vt")
                    nc.tensor.transpose(
                        ps_t[:], vst[hp][:, tb * 128:(tb + 1) * 128],
                        ident_t[:])
                    # copies on ACT: DVE is the phase-A bottleneck
                    nc.scalar.activation(v_pad[2 * hp][:, tb, 0:64],
                                         ps_t[:, 0:64], AF.Copy)
                    nc.scalar.activation(v_pad[2 * hp + 1][:, tb, 64:128],
                                         ps_t[:, 64:128], AF.Copy)

                vt_done = 0
                for ci, ct in enumerate(CT_ORDER):
                    ms = slice(ct * 128, (ct + 1) * 128)
                    for t in range(NT):
                        cs = slice(t * 512, (t + 1) * 512)
                        ps_m = psam.tile([128, 512], F32, tag="qkv")
                        for k in range(NK):
                            nc.tensor.matmul(ps_m[:], wq[:, k, ms],
                                             xT[:, k, cs], start=(k == 0),
                                             stop=False, skip_group_check=True)
                        # LN corrections: u*(-mean) + vb*std as one K=2 matmul
                        nc.tensor.matmul(ps_m[:], uv_t[0:2, ms],
                                         rows[0:2, cs], start=False,
                                         stop=True, skip_group_check=True)
                        if ct in (0, 1):      # q
                            pair = ct
                            nc.vector.tensor_tensor(qT[pair][:, cs], ps_m[:],
                                                    a_bc[:, cs], op=MUL)
                        elif ct in (4, 5):    # v -> staging
                            hp = ct - 4
                            nc.vector.tensor_tensor(vst[hp][:, cs], ps_m[:],
                                                    a_bc[:, cs], op=MUL)
                        else:                 # k -> zero-padded halves
                            pair = ct - 2
                            nc.vector.tensor_tensor(
                                kTz[2 * pair][0:64, cs], ps_m[0:64, :],
                                a_bc[0:64, cs], op=MUL)
                            nc.vector.tensor_tensor(
                                kTz[2 * pair + 1][64:128, cs],
                                ps_m[64:128, :], a_bc[64:128, cs], op=MUL)
                    if ci >= 1 and vt_done < 32:
                        # after v23: 8 transposes per completed ct
                        for _ in range(8):
                            hp, tb = divmod(vt_done, NROW)
                            _vtrans(hp, tb)
                            vt_done += 1

            # ---------------- phase C+D: attention + fused out-proj --------
            with (
                tc.tile_pool(name="pC", bufs=4) as pc,
                tc.tile_pool(name="pCd", bufs=3) as pcd,
                tc.tile_pool(name="pD", bufs=3) as pd,
                tc.tile_pool(name="psS", bufs=3, space="PSUM") as pss,
                tc.tile_pool(name="psO", bufs=1, space="PSUM") as pso,
            ):
                # deferred out-projection slabs: emitted interleaved into the
                # NEXT block's j-loop so the PE has filler work during the
                # exp-bound steady state (keeps HAM activity high = warm)
                deferred = []

                op_n = [0]

                def _emit_op_slab(rsl):
                    op_ps = pss.tile([128, 1024], F32, tag="sp")
                    for mt in range(2):
                        msl = slice(mt * 512, (mt + 1) * 512)
                        nc.tensor.matmul(op_ps[:, msl], oT[0][:, rsl],
                                         wo_t[:, 0, msl], start=True,
                                         stop=False, skip_group_check=True)
                        nc.tensor.matmul(op_ps[:, msl], oT[1][:, rsl],
                                         wo_t[:, 1, msl], start=False,
                                         stop=True, skip_group_check=True)
                    ost = pd.tile([128, 1024], BF16, tag="ost")
                    if op_n[0] % 2 == 0:
                        nc.vector.tensor_copy(ost[:], op_ps[:])
                    else:
                        nc.scalar.activation(ost[:], op_ps[:], AF.Copy)
                    op_n[0] += 1
                    nc.sync.dma_start(out_ext[rsl, :], ost[:])

                for ib in (3, 2, 1, 0):
                    i0 = ib * 512
                    isl = slice(i0, i0 + 512)
                    n_jt = 4 * (ib + 1)
                    for pair in range(2):
                        o0 = pso.tile([128, 512], F32, tag="o0")
                        o1 = pso.tile([128, 512], F32, tag="o1")
                        pvq = []

                        def _emit_pv(ent, last, o0=o0, o1=o1, pair=pair):
                            jt, so, e_t = ent
                            nc.tensor.matmul(
                                o0[:, so:512], v_pad[2 * pair][:, jt, :],
                                e_t[:, so:512], start=(jt == 0), stop=last,
                                skip_group_check=True)
                            nc.tensor.matmul(
                                o1[:, so:512], v_pad[2 * pair + 1][:, jt, :],
                                e_t[:, 512 + so:1024], start=(jt == 0),
                                stop=last, skip_group_check=True)

                        for jt in range(n_jt):
                            j0 = jt * 128
                            so = max(0, j0 - i0)
                            diag = j0 >= i0
                            s_ps = pss.tile([128, 1024], F32, tag="sp")
                            nc.tensor.matmul(
                                s_ps[:, so:512],
                                kTz[2 * pair][:, j0:j0 + 128],
                                qT[pair][:, i0 + so:i0 + 512],
                                start=True, stop=not diag,
                                skip_group_check=True)
                            nc.tensor.matmul(
                                s_ps[:, 512 + so:1024],
                                kTz[2 * pair + 1][:, j0:j0 + 128],
                                qT[pair][:, i0 + so:i0 + 512],
                                start=True, stop=not diag,
                                skip_group_check=True)
                            if diag:
                                # causal mask: accumulate -30000*[c < j] via
                                # (-30000*triu(k<j)).T @ I — stays on the PE,
                                # no extra engine hop before the exp
                                nc.tensor.matmul(
                                    s_ps[:, so:so + 128], stri_t[:],
                                    ident_t[:], start=False, stop=True,
                                    skip_group_check=True)
                                nc.tensor.matmul(
                                    s_ps[:, 512 + so:512 + so + 128],
                                    stri_t[:], ident_t[:], start=False,
                                    stop=True, skip_group_check=True)
                            e_t = pc.tile([128, 1024], BF16, tag="e")
                            if so == 0:
                                nc.scalar.activation(e_t[:], s_ps[:], AF.Exp)
                            else:
                                sv = s_ps[:].rearrange("p (h n) -> p h n",
                                                       h=2)[:, :, so:512]
                                ev = e_t[:].rearrange("p (h n) -> p h n",
                                                      h=2)[:, :, so:512]
                                nc.scalar.activation(ev, sv, AF.Exp)
                            pvq.append((jt, so, e_t))
                            # drain eagerly near loop end so the last PV (and
                            # with it the normalize chain) lands sooner
                            lag = 3 if jt < n_jt - 2 else 1
                            while len(pvq) > lag:
                                _emit_pv(pvq.pop(0), last=False)
                            if jt % 2 == 1 and deferred:
                                deferred.pop(0)()
                            if ib <= 2:
                                # benign weight-load filler: streams the PE
                                # array during exp-bound idle so the HAM
                                # activity monitor keeps the 2.4 GHz clock
                                nc.tensor.ldweights(
                                    kTz[2 * pair][:, j0:j0 + 128])
                        while pvq:
                            _emit_pv(pvq.pop(0), last=(len(pvq) == 0))

                        # normalize: denominators at o0 row 64 / o1 row 0.
                        dn = pcd.tile([65, 512], F32R, tag="dn")
                        nc.scalar.activation(dn[64:65, :], o0[64:65, :],
                                             AF.Copy)
                        nc.scalar.activation(dn[0:1, :], o1[0:1, :], AF.Copy)
                        rb = pss.tile([128, 1024], F32, tag="sp")
                        nc.tensor.matmul(rb[:, 0:512], ones_t[64:65, :],
                                         dn[64:65, :],
                                         start=True, stop=True,
                                         skip_group_check=True)
                        nc.tensor.matmul(rb[:, 512:1024], ones_t[0:1, :],
                                         dn[0:1, :],
                                         start=True, stop=True,
                                         skip_group_check=True)
                        # custom DVE recip: full-tile base-0 ops only
                        rdb = pcd.tile([128, 1024], F32, tag="rdb")
                        nc.vector.reciprocal_approx_fast(rdb[:], rb[:])
                        nc.vector.tensor_tensor(oT[pair][0:64, isl],
                                                o0[0:64, :], rdb[0:64, 0:512],
                                                op=MUL)
                        nc.vector.tensor_tensor(oT[pair][64:128, isl],
                                                o1[64:128, :],
                                                rdb[64:128, 512:1024], op=MUL)

                    # out-projection for this block: defer into the next
                    # block's j-loop (last block drains immediately below)
                    for ts4 in range(4):
                        rsl = slice(i0 + ts4 * 128, i0 + (ts4 + 1) * 128)
                        deferred.append(
                            lambda rsl=rsl: _emit_op_slab(rsl))
                while deferred:
                    deferred.pop(0)()

    nc.compile()
    return nc


_NC_CACHE = {}


def _get_nc():
    if "nc" not in _NC_CACHE:
        _NC_CACHE["nc"] = _build()
    return _NC_CACHE["nc"]


def _prep_in_maps(x, ln_w, ln_b, w_qkv, w_out):
    import ml_dtypes
    _bf = ml_dtypes.bfloat16
    x = np.asarray(x, dtype=np.float32)
    ln_w = np.asarray(ln_w, dtype=np.float32)
    ln_b = np.asarray(ln_b, dtype=np.float32)
    w_qkv = np.asarray(w_qkv, dtype=np.float32)
    w_out = np.asarray(w_out, dtype=np.float32)

    ones = np.ones((128, 128), dtype=np.float32)
    # stri[k, j] = -30000 iff k < j; (stri.T @ I)[j, c] = -30000*[c < j],
    # the additive causal mask accumulated into the diagonal score tiles
    stri = (-30000.0 * np.triu(np.ones((128, 128), dtype=np.float32), k=1)
            ).astype(ml_dtypes.bfloat16)
    ident = np.eye(128, dtype=np.float32)

    xTs = [x[b].T.astype(_bf) for b in range(B)]

    in_maps = []
    for core in range(8):
        b, hg = core // 4, core % 4
        csl = slice(hg * CD, (hg + 1) * CD)
        # raw slices with SCALE folded into q
        w0 = np.concatenate([w_qkv[:, csl] * SCALE,
                             w_qkv[:, DIM + hg * CD:DIM + (hg + 1) * CD],
                             w_qkv[:, 2 * DIM + hg * CD:2 * DIM + (hg + 1) * CD]],
                            axis=1)
        wf = ln_w[:, None] * w0                      # ln_w folded
        u = wf.sum(axis=0)                           # pairs with -mean
        vb = ln_b @ w0                               # pairs with std (ln bias)
        uv = np.stack([u, vb]).astype(np.float32)
        in_maps.append({
            "xT": xTs[b],
            "wqkv": wf.astype(_bf),
            "uv": _r32(uv),
            "wout": w_out[csl, :].astype(_bf),
            "ones": ones,
            "stri": stri,
            "ident": ident.astype(_bf),
        })
    return in_maps


def _combine(results):
    out = np.empty((B, N, DIM), dtype=np.float32)
    for b in range(B):
        acc = results[b * 4]["out"].astype(np.float32)
        for hg in range(1, 4):
            acc = acc + results[b * 4 + hg]["out"].astype(np.float32)
        out[b] = acc
    return out


def kernel(x, ln_w, ln_b, w_qkv, w_out):
    nc = _get_nc()
    in_maps = _prep_in_maps(x, ln_w, ln_b, w_qkv, w_out)
    res = run_bass_kernel_spmd(nc, in_maps, core_ids=list(range(8)))
    return _combine(res.results)


def run_traced(x, ln_w, ln_b, w_qkv, w_out, **kwargs):
    """Run with NTFF profiling; returns (output, BassKernelResults)."""
    nc = _get_nc()
    in_maps = _prep_in_maps(x, ln_w, ln_b, w_qkv, w_out)
    res = run_bass_kernel_spmd(nc, in_maps, core_ids=list(range(8)),
                               trace=True, **kwargs)
    return _combine(res.results), res


# revision 38
# speedup vs baseline: 1.0010x; 1.0010x over previous
"""Fused LayerNorm + causal multi-head attention for Trainium2, 8 NeuronCores.

Problem: x[2,2048,1024] -> LN -> qkv proj (w_qkv[1024,3072]) -> 16-head causal
attention (d=64) -> out proj (w_out[1024,1024]).

Sharding (no cross-core communication):
  core c = b*4 + hg   (b in {0,1} batches, hg in {0..3} head-groups of 4 heads)
  Each core computes its batch's LN + its 4 heads' qkv/attention + a partial
  out-projection (its 256 rows of w_out). Host sums the 4 partials per batch.

Perf design (v2, ~231us vs 358us baseline): every matmul is shaped "fat"
(K=128 streamed rows, M<=128 with padding) so the PE HAM clock gate stays at
2.4 GHz (the baseline's attention ran entirely at the cold 1.2 GHz clock
because its K=64/M=66 matmuls never register enough array activity):
  - s = k.T @ q uses zero-padded k tiles (kTz): head-even data on partitions
    0:64 with zeros below, head-odd data on 64:128 with zeros above, so the
    shared q tile (both heads stacked) streams through a K=128 matmul.
  - PV uses v_pad[128,128] stationary tiles: head-even v at cols 0:64 with a
    ones col at 64 (softmax denominator lands on psum row 64); head-odd v at
    cols 64:128 with a ones col at 0 (denominator on psum row 0). Outputs land
    lane-aligned with the oT layout (even head rows 0:64, odd rows 64:128), so
    normalization is pure DVE (no partition-shift DMA).
  - causal masks are ADDITIVE pre-exp, accumulated into the score psum by a
    (-30000*triu).T @ I matmul: the mask stays inside the PE stream instead
    of adding a GpSimd hop between exp and PV.
  - i-blocks run in order 3,2,1,0 and each block's out-projection is deferred
    and drip-fed into the NEXT block's j-loop as PE filler: the exp-bound
    steady state would otherwise leave ~25% distributed PE idle, which trips
    the HAM activity monitor back to 1.2 GHz.
  - exp is one [128,1024] ACT instruction per head-pair j-tile (2-bank psum
    read); bf16 out-projection output halves the store DMA bytes.
PSUM (8 banks): s/rb/outproj share one 3-buf [128,1024] pool (6) + o0/o1 (2).
Known HW constraints honored: reciprocal_approx_fast must be base-partition-0
and not in-place; ACT ops need quad-aligned partition bases; f32r matmul
operands must be produced as f32r (rounded) by the writing op; K=1 f32r
matmuls only from partition 0/64; GPSIMD cannot touch PSUM.
"""
import os
import sys

for _p in ("/opt/trn_rl_repo", "/root/.axon_site/_ro/trn_rl_repo"):
    if os.path.isdir(_p) and _p not in sys.path:
        sys.path.insert(0, _p)

import numpy as np

import concourse.bass as bass  # noqa: F401  (import side effects / debugging)
import concourse.mybir as mybir
import concourse.tile as tile
from concourse import bacc
from concourse.bass_utils import run_bass_kernel_spmd

F32 = mybir.dt.float32
F32R = mybir.dt.float32r
BF16 = mybir.dt.bfloat16
MUL = mybir.AluOpType.mult
ADD = mybir.AluOpType.add
SUB = mybir.AluOpType.subtract
AF = mybir.ActivationFunctionType

B, N, DIM = 2, 2048, 1024
HEADS, DH = 16, 64
HPC = 4            # heads per core
CD = HPC * DH      # 256 output channels per core
SCALE = DH ** -0.5
EPS = 1e-5
NT = N // 512      # 4 col-blocks of 512
NK = DIM // 128    # 8 contraction chunks
NROW = N // 128    # 16 row tiles of 128

# w layout cols: [q01 q23 k01 k23 v01 v23] blocks of 128; emit v first so the
# PE-transposes interleave with later q/k matmuls (keeps HAM warm).
CT_V = (4, 5)
CT_ORDER = (4, 5, 0, 2, 1, 3)   # v01 v23 q01 k01 q23 k23


def _r32(x, bits=13):
    """tf32-style rounding: round-to-nearest, drop low `bits` mantissa bits."""
    v = np.ascontiguousarray(x, dtype=np.float32).view(np.uint32)
    v = (v + (1 << (bits - 1))) & np.uint32(~((1 << bits) - 1) & 0xFFFFFFFF)
    return v.view(np.float32)


def _build():
    nc = bacc.Bacc("TRN2", target_bir_lowering=False, debug=False)

    xT_ext = nc.declare_dram_parameter("xT", [DIM, N], BF16, isOutput=False)
    w_ext = nc.declare_dram_parameter("wqkv", [DIM, 3 * CD], BF16, isOutput=False)
    uv_ext = nc.declare_dram_parameter("uv", [2, 3 * CD], F32R, isOutput=False)
    wo_ext = nc.declare_dram_parameter("wout", [CD, DIM], BF16, isOutput=False)
    ones_ext = nc.declare_dram_parameter("ones", [128, 128], F32R, isOutput=False)
    stri_ext = nc.declare_dram_parameter("stri", [128, 128], BF16, isOutput=False)
    id_ext = nc.declare_dram_parameter("ident", [128, 128], BF16, isOutput=False)
    out_ext = nc.declare_dram_parameter("out", [N, DIM], BF16, isOutput=True)

    with tile.TileContext(nc) as tc:
        with (
            nc.allow_low_precision(reason="float32r is 4-byte; psum stays f32"),
            tc.tile_pool(name="persist", bufs=1) as pp,
            tc.tile_pool(name="small", bufs=1) as sp,
        ):
            ones_t = pp.tile([128, 128], F32R, tag="ones")
            stri_t = pp.tile([128, 128], BF16, tag="stri")
            ident_t = pp.tile([128, 128], BF16, tag="ident")
            nc.sync.dma_start(ones_t[:], ones_ext[:])
            nc.sync.dma_start(stri_t[:], stri_ext[:])
            nc.sync.dma_start(ident_t[:], id_ext[:])
            ones_b = pp.tile([128, 128], BF16, tag="ones_b")
            nc.vector.tensor_scalar(ones_b[:], ones_t[:], 0.0, 1.0,
                                    op0=MUL, op1=ADD)

            # persistent SBUF tensors
            qT = [pp.tile([128, N], BF16, tag=f"qT{p}", name=f"qT{p}")
                  for p in range(2)]                      # q pair tiles
            vst = [pp.tile([128, N], BF16, tag=f"vst{p}", name=f"vst{p}")
                   for p in range(2)]                     # v staging (dh-major)
            kTz = [pp.tile([128, N], BF16, tag=f"kTz{h}", name=f"kTz{h}")
                   for h in range(4)]                     # zero-padded k tiles
            v_pad = [pp.tile([128, NROW, 128], BF16, tag=f"vp{h}",
                             name=f"vp{h}") for h in range(4)]
            oT = [pp.tile([128, N], BF16, tag=f"oT{p}", name=f"oT{p}")
                  for p in range(2)]
            a_bc = pp.tile([128, N], F32R, tag="a_bc")    # rs[n] broadcast
            rows = sp.tile([66, N], F32R, tag="rows")     # p0=-mean p1=std

            # ---------------- phase A: stats + qkv + v transpose ----------
            with (
                tc.tile_pool(name="pA", bufs=1) as pa,
                tc.tile_pool(name="pAx", bufs=2) as pax,
                tc.tile_pool(name="psA", bufs=1, space="PSUM") as psa,
                tc.tile_pool(name="psAm", bufs=3, space="PSUM") as psam,
                tc.tile_pool(name="psAv", bufs=2, space="PSUM") as psav,
            ):
                xT = pa.tile([128, NK, N], BF16, tag="xT")
                wq = pa.tile([128, NK, 3 * CD], BF16, tag="wq")
                uv_t = pa.tile([2, 3 * CD], F32R, tag="uv")
                wo_t = pa.tile([128, 2, DIM], BF16, tag="wo")
                nc.sync.dma_start(uv_t[:], uv_ext[:])
                xT_d = xT_ext[:].rearrange("(c p) n -> p c n", p=128)
                w_d = w_ext[:].rearrange("(c p) m -> p c m", p=128)
                # x loads ordered so stats(t=0) can start after the first 1 MB
                for k in range(NK):
                    nc.sync.dma_start(xT[:, k, 0:512], xT_d[:, k, 0:512])
                for k in range(NK):
                    nc.sync.dma_start(xT[:, k, 512:1024], xT_d[:, k, 512:1024])
                for k in range(NK):
                    nc.sync.dma_start(wq[:, k, :], w_d[:, k, :])
                for k in range(NK):
                    nc.sync.dma_start(xT[:, k, 1024:2048], xT_d[:, k, 1024:2048])
                wo_d = wo_ext[:].rearrange("(c p) m -> p c m", p=128)
                nc.sync.dma_start(wo_t[:, 0, :], wo_d[:, 0, :])
                nc.sync.dma_start(wo_t[:, 1, :], wo_d[:, 1, :])

                # stats per 512-col block: colsums of x and x^2 replicated to
                # all partitions via ones-matmuls; then mean/std/rs rows.
                # All row-chain ops run on partitions 0:66 (data replicated)
                # so each needed row (p0, p1, p64) is produced in-lane.
                for t in range(NT):
                    cs = slice(t * 512, (t + 1) * 512)
                    ps_s = psa.tile([128, 512], F32, tag="st_s")
                    ps_q = psa.tile([128, 512], F32, tag="st_q")
                    for k in range(NK):
                        xsq = pax.tile([128, 512], BF16, tag="xsq")
                        eng = nc.vector if k % 2 == 0 else nc.gpsimd
                        eng.tensor_tensor(xsq[:], xT[:, k, cs],
                                          xT[:, k, cs], op=MUL)
                        nc.tensor.matmul(ps_s[:], ones_b[:], xT[:, k, cs],
                                         start=(k == 0), stop=(k == NK - 1),
                                         skip_group_check=True)
                        nc.tensor.matmul(ps_q[:], ones_b[:], xsq[:],
                                         start=(k == 0), stop=(k == NK - 1),
                                         skip_group_check=True)
                    scm = pax.tile([66, 512], F32, tag="scm")
                    scq = pax.tile([66, 512], F32, tag="scq")
                    scv = pax.tile([66, 512], F32, tag="scv")
                    nc.vector.tensor_scalar(scm[:], ps_s[0:66, :], 1.0 / DIM,
                                            None, op0=MUL)
                    nc.vector.tensor_scalar(scq[:], ps_q[0:66, :], 1.0 / DIM,
                                            EPS, op0=MUL, op1=ADD)
                    nc.vector.tensor_tensor(scv[:], scm[:], scm[:], op=MUL)
                    nc.vector.tensor_tensor(scv[:], scq[:], scv[:], op=SUB)
                    # std rows at p0:2 (ACT base must be quad-aligned), then
                    # overwrite p0 with -mean: rows = [-mean@p0, std@p1]
                    nc.scalar.activation(rows[0:2, cs], scv[0:2, :], AF.Sqrt)
                    nc.vector.tensor_scalar(rows[0:1, cs], scm[0:1, :], -1.0,
                                            None, op0=MUL)
                    # rs = 1/std at p0, broadcast to all partitions via K=1.
                    # (reciprocal_approx_fast is a custom DVE op: base
                    # partition must be 0 and it must NOT run in-place.)
                    scs = pax.tile([1, 512], F32, tag="scs")
                    scs2 = pax.tile([1, 512], F32, tag="scs2")
                    scr_r = pax.tile([1, 512], F32R, tag="scr_r")
                    nc.scalar.activation(scs[0:1, :], scv[0:1, :], AF.Sqrt)
                    nc.vector.reciprocal_approx_fast(scs2[0:1, :],
                                                     scs[0:1, :])
                    nc.vector.tensor_copy(scr_r[0:1, :], scs2[0:1, :])
                    ps_ab = psa.tile([128, 512], F32, tag="ab")
                    nc.tensor.matmul(ps_ab[:], ones_t[0:1, :],
                                     scr_r[0:1, :], start=True,
                                     stop=True, skip_group_check=True)
                    nc.vector.tensor_copy(a_bc[:, cs], ps_ab[:])

                # padded-tile pre-fill AFTER the stats loop: gpsimd executes
                # in order, and ~14us of memsets ahead of the stats squares
                # would stall the x^2 colsum chain at kernel start.
                # kTz[2p]: head-even k on partitions 0:64, zeros below;
                # kTz[2p+1]: head-odd k on 64:128, zeros above.
                # v_pad[2p]: v at cols 0:64, ones col 64, zeros 65:128;
                # v_pad[2p+1]: ones col 0, zeros 1:64, v at cols 64:128.
                for p in range(2):
                    nc.gpsimd.memset(kTz[2 * p][64:128, :], 0.0)
                    nc.gpsimd.memset(kTz[2 * p + 1][0:64, :], 0.0)
                for h in range(4):
                    nc.gpsimd.memset(v_pad[h][:], 0.0)
                for p in range(2):
                    nc.gpsimd.memset(v_pad[2 * p][:, :, 64:65], 1.0)
                    nc.gpsimd.memset(v_pad[2 * p + 1][:, :, 0:1], 1.0)

                # qkv projection; v cts first, then q/k with the 32 v
                # transposes interleaved (8 after each ct) to keep PE dense.
                def _vtrans(hp, tb):
                    ps_t = psav.tile([128, 128], BF16, tag="vt")
                    nc.tensor.transpose(
                        ps_t[:], vst[hp][:, tb * 128:(tb + 1) * 128],
                        ident_t[:])
                    # copies on ACT: DVE is the phase-A bottleneck
                    nc.scalar.activation(v_pad[2 * hp][:, tb, 0:64],
                                         ps_t[:, 0:64], AF.Copy)
                    nc.scalar.activation(v_pad[2 * hp + 1][:, tb, 64:128],
                                         ps_t[:, 64:128], AF.Copy)

                vt_done = 0
                for ci, ct in enumerate(CT_ORDER):
                    ms = slice(ct * 128, (ct + 1) * 128)
                    for t in range(NT):
                        cs = slice(t * 512, (t + 1) * 512)
                        ps_m = psam.tile([128, 512], F32, tag="qkv")
                        for k in range(NK):
                            nc.tensor.matmul(ps_m[:], wq[:, k, ms],
                                             xT[:, k, cs], start=(k == 0),
                                             stop=False, skip_group_check=True)
                        # LN corrections: u*(-mean) + vb*std as one K=2 matmul
                        nc.tensor.matmul(ps_m[:], uv_t[0:2, ms],
                                         rows[0:2, cs], start=False,
                                         stop=True, skip_group_check=True)
                        if ct in (0, 1):      # q
                            pair = ct
                            nc.vector.tensor_tensor(qT[pair][:, cs], ps_m[:],
                                                    a_bc[:, cs], op=MUL)
                        elif ct in (4, 5):    # v -> staging
                            hp = ct - 4
                            nc.vector.tensor_tensor(vst[hp][:, cs], ps_m[:],
                                                    a_bc[:, cs], op=MUL)
                        else:                 # k -> zero-padded halves
                            pair = ct - 2
                            nc.vector.tensor_tensor(
                                kTz[2 * pair][0:64, cs], ps_m[0:64, :],
                                a_bc[0:64, cs], op=MUL)
                            nc.vector.tensor_tensor(
                                kTz[2 * pair + 1][64:128, cs],
                                ps_m[64:128, :], a_bc[64:128, cs], op=MUL)
                    if ci >= 1 and vt_done < 32:
                        # after v23: 8 transposes per completed ct
                        for _ in range(8):
                            hp, tb = divmod(vt_done, NROW)
                            _vtrans(hp, tb)
                            vt_done += 1

            # ---------------- phase C+D: attention + fused out-proj --------
            with (
                tc.tile_pool(name="pC", bufs=4) as pc,
                tc.tile_pool(name="pCd", bufs=3) as pcd,
                tc.tile_pool(name="pD", bufs=3) as pd,
                tc.tile_pool(name="psS", bufs=3, space="PSUM") as pss,
                tc.tile_pool(name="psO", bufs=1, space="PSUM") as pso,
            ):
                # deferred out-projection slabs: emitted interleaved into the
                # NEXT block's j-loop so the PE has filler work during the
                # exp-bound steady state (keeps HAM activity high = warm)
                deferred = []

                op_n = [0]

                def _emit_op_slab(rsl):
                    op_ps = pss.tile([128, 1024], F32, tag="sp")
                    for mt in range(2):
                        msl = slice(mt * 512, (mt + 1) * 512)
                        nc.tensor.matmul(op_ps[:, msl], oT[0][:, rsl],
                                         wo_t[:, 0, msl], start=True,
                                         stop=False, skip_group_check=True)
                        nc.tensor.matmul(op_ps[:, msl], oT[1][:, rsl],
                                         wo_t[:, 1, msl], start=False,
                                         stop=True, skip_group_check=True)
                    ost = pd.tile([128, 1024], BF16, tag="ost")
                    if op_n[0] % 2 == 0:
                        nc.vector.tensor_copy(ost[:], op_ps[:])
                    else:
                        nc.scalar.activation(ost[:], op_ps[:], AF.Copy)
                    op_n[0] += 1
                    nc.sync.dma_start(out_ext[rsl, :], ost[:])

                for ib in (3, 2, 1, 0):
                    i0 = ib * 512
                    isl = slice(i0, i0 + 512)
                    n_jt = 4 * (ib + 1)
                    for pair in range(2):
                        o0 = pso.tile([128, 512], F32, tag="o0")
                        o1 = pso.tile([128, 512], F32, tag="o1")
                        pvq = []

                        def _emit_pv(ent, last, o0=o0, o1=o1, pair=pair):
                            jt, so, e_t = ent
                            nc.tensor.matmul(
                                o0[:, so:512], v_pad[2 * pair][:, jt, :],
                                e_t[:, so:512], start=(jt == 0), stop=last,
                                skip_group_check=True)
                            nc.tensor.matmul(
                                o1[:, so:512], v_pad[2 * pair + 1][:, jt, :],
                                e_t[:, 512 + so:1024], start=(jt == 0),
                                stop=last, skip_group_check=True)

                        for jt in range(n_jt):
                            j0 = jt * 128
                            so = max(0, j0 - i0)
                            diag = j0 >= i0
                            s_ps = pss.tile([128, 1024], F32, tag="sp")
                            nc.tensor.matmul(
                                s_ps[:, so:512],
                                kTz[2 * pair][:, j0:j0 + 128],
                                qT[pair][:, i0 + so:i0 + 512],
                                start=True, stop=not diag,
                                skip_group_check=True)
                            nc.tensor.matmul(
                                s_ps[:, 512 + so:1024],
                                kTz[2 * pair + 1][:, j0:j0 + 128],
                                qT[pair][:, i0 + so:i0 + 512],
                                start=True, stop=not diag,
                                skip_group_check=True)
                            if diag:
                                # causal mask: accumulate -30000*[c < j] via
                                # (-30000*triu(k<j)).T @ I — stays on the PE,
                                # no extra engine hop before the exp
                                nc.tensor.matmul(
                                    s_ps[:, so:so + 128], stri_t[:],
                                    ident_t[:], start=False, stop=True,
                                    skip_group_check=True)
                                nc.tensor.matmul(
                                    s_ps[:, 512 + so:512 + so + 128],
                                    stri_t[:], ident_t[:], start=False,
                                    stop=True, skip_group_check=True)
                            e_t = pc.tile([128, 1024], BF16, tag="e")
                            if so == 0:
                                nc.scalar.activation(e_t[:], s_ps[:], AF.Exp)
                            else:
                                sv = s_ps[:].rearrange("p (h n) -> p h n",
                                                       h=2)[:, :, so:512]
                                ev = e_t[:].rearrange("p (h n) -> p h n",
                                                      h=2)[:, :, so:512]
                                nc.scalar.activation(ev, sv, AF.Exp)
                            pvq.append((jt, so, e_t))
                            if len(pvq) > 3:
                                _emit_pv(pvq.pop(0), last=False)
                            if jt % 2 == 1 and deferred:
                                deferred.pop(0)()
                        while pvq:
                            _emit_pv(pvq.pop(0), last=(len(pvq) == 0))

                        # normalize: denominators at o0 row 64 / o1 row 0.
                        dn = pcd.tile([65, 512], F32R, tag="dn")
                        nc.scalar.activation(dn[64:65, :], o0[64:65, :],
                                             AF.Copy)
                        nc.scalar.activation(dn[0:1, :], o1[0:1, :], AF.Copy)
                        rb = pss.tile([128, 1024], F32, tag="sp")
                        nc.tensor.matmul(rb[:, 0:512], ones_t[64:65, :],
                                         dn[64:65, :],
                                         start=True, stop=True,
                                         skip_group_check=True)
                        nc.tensor.matmul(rb[:, 512:1024], ones_t[0:1, :],
                                         dn[0:1, :],
                                         start=True, stop=True,
                                         skip_group_check=True)
                        # custom DVE recip: full-tile base-0 ops only
                        rdb = pcd.tile([128, 1024], F32, tag="rdb")
                        nc.vector.reciprocal_approx_fast(rdb[:], rb[:])
                        nc.vector.tensor_tensor(oT[pair][0:64, isl],
                                                o0[0:64, :], rdb[0:64, 0:512],
                                                op=MUL)
                        nc.vector.tensor_tensor(oT[pair][64:128, isl],
                                                o1[64:128, :],
                                                rdb[64:128, 512:1024], op=MUL)

                    # out-projection for this block: defer into the next
                    # block's j-loop (last block drains immediately below)
                    for ts4 in range(4):
                        rsl = slice(i0 + ts4 * 128, i0 + (ts4 + 1) * 128)
                        deferred.append(
                            lambda rsl=rsl: _emit_op_slab(rsl))
                while deferred:
                    deferred.pop(0)()

    nc.compile()
    return nc


_NC_CACHE = {}


def _get_nc():
    if "nc" not in _NC_CACHE:
        _NC_CACHE["nc"] = _build()
    return _NC_CACHE["nc"]


def _prep_in_maps(x, ln_w, ln_b, w_qkv, w_out):
    import ml_dtypes
    _bf = ml_dtypes.bfloat16
    x = np.asarray(x, dtype=np.float32)
    ln_w = np.asarray(ln_w, dtype=np.float32)
    ln_b = np.asarray(ln_b, dtype=np.float32)
    w_qkv = np.asarray(w_qkv, dtype=np.float32)
    w_out = np.asarray(w_out, dtype=np.float32)

    ones = np.ones((128, 128), dtype=np.float32)
    # stri[k, j] = -30000 iff k < j; (stri.T @ I)[j, c] = -30000*[c < j],
    # the additive causal mask accumulated into the diagonal score tiles
    stri = (-30000.0 * np.triu(np.ones((128, 128), dtype=np.float32), k=1)
            ).astype(ml_dtypes.bfloat16)
    ident = np.eye(128, dtype=np.float32)

    xTs = [x[b].T.astype(_bf) for b in range(B)]

    in_maps = []
    for core in range(8):
        b, hg = core // 4, core % 4
        csl = slice(hg * CD, (hg + 1) * CD)
        # raw slices with SCALE folded into q
        w0 = np.concatenate([w_qkv[:, csl] * SCALE,
                             w_qkv[:, DIM + hg * CD:DIM + (hg + 1) * CD],
                             w_qkv[:, 2 * DIM + hg * CD:2 * DIM + (hg + 1) * CD]],
                            axis=1)
        wf = ln_w[:, None] * w0                      # ln_w folded
        u = wf.sum(axis=0)                           # pairs with -mean
        vb = ln_b @ w0                               # pairs with std (ln bias)
        uv = np.stack([u, vb]).astype(np.float32)
        in_maps.append({
            "xT": xTs[b],
            "wqkv": wf.astype(_bf),
            "uv": _r32(uv),
            "wout": w_out[csl, :].astype(_bf),
            "ones": ones,
            "stri": stri,
            "ident": ident.astype(_bf),
        })
    return in_maps


def _combine(results):
    out = np.empty((B, N, DIM), dtype=np.float32)
    for b in range(B):
        acc = results[b * 4]["out"].astype(np.float32)
        for hg in range(1, 4):
            acc = acc + results[b * 4 + hg]["out"].astype(np.float32)
        out[b] = acc
    return out


def kernel(x, ln_w, ln_b, w_qkv, w_out):
    nc = _get_nc()
    in_maps = _prep_in_maps(x, ln_w, ln_b, w_qkv, w_out)
    res = run_bass_kernel_spmd(nc, in_maps, core_ids=list(range(8)))
    return _combine(res.results)


def run_traced(x, ln_w, ln_b, w_qkv, w_out, **kwargs):
    """Run with NTFF profiling; returns (output, BassKernelResults)."""
    nc = _get_nc()
    in_maps = _prep_in_maps(x, ln_w, ln_b, w_qkv, w_out)
    res = run_bass_kernel_spmd(nc, in_maps, core_ids=list(range(8)),
                               trace=True, **kwargs)
    return _combine(res.results), res
